# revision 1
# baseline (speedup 1.0000x reference)
"""Trainium2 Bass kernel for nn_CustomSelfAttention_24257975288159.

Reference computation (B=4, L=2048, D=1024, H=16, HD=64, fp32):
  q = x @ Wq + bq ; q[:, 1, :] = cross_cls_sent @ Wq + bq
  k = x @ Wk + bk ; v = x @ Wv + bv
  out = softmax(q k^T / sqrt(HD) + mask) v       (per head)

Sharding: 8 cores = batch (4) x head-group (2). Core c handles batch
c//2 and heads 8*(c%2)..8*(c%2)+7, i.e. columns 512*(c%2)..+512 of the
model dim; QKV weights are column-sharded per head group.

Per-core device algorithm (matmuls in float32r, ~1e-4 rel err):
  1. PE-transpose x into xT tiles [d-part, l-free].
  2. Projections: qT,kT in [head-dim-part, l-free] layout (W chunk as
     stationary, xT as moving); v in natural [l-part, dh-free] layout
     (xT chunk stationary, Wv moving) with a ones column appended per
     head so the ctx matmul also produces softmax denominators.
  3. Attention per head-pair, per lq quarter: transposed scores
     sT[lk-part, lq-free] via row-tiled matmul pairs (K=64 in each half
     of the PE array, both heads into one [128,1024] psum tile), one
     fused exp(SCALE*s + mask) on ScalarE covering both heads, then
     M=65 ctx matmuls accumulating ctx^T plus the denominator row.
     Scores run two chunks ahead of exp (software pipeline); the next
     pair's projections are emitted before each attention block so the
     PE fills ACT-bound stretches.
  4. Finish per pair: PE-transpose denominators + ctx^T back to
     [lq-part, dh-free], multiply by reciprocal denominators, DMA the
     128-column strip out.
"""
import numpy as np

import concourse.bass as bass
import concourse.mybir as mybir
import concourse.tile as tile
from concourse.masks import make_identity

F32 = mybir.dt.float32
F32R = mybir.dt.float32r

B, L, D, H = 4, 2048, 1024, 16
HD = D // H          # 64
SCALE = float(1.0 / np.sqrt(HD))
DG = D // 2          # 512 output columns per core (8 heads)
NCORES = 8
LC = L // 128        # 16 l-chunks
DC = D // 128        # 8 d-chunks
GC = DG // 128       # 4 dh-chunks per core = head pairs
HS = HD + 1          # 65: v columns per head incl ones column

_CACHED = {}


# ---------------------------------------------------------------------------
# walrus workaround: this build rejects >1 sync-wait per instruction.
# Spill excess waits onto single-wait NOPs on the same engine.
# ---------------------------------------------------------------------------
def _split_excess_waits(nc, max_waits=1):
    counter = 0
    for fn in nc.m.functions:
        for blk in fn.blocks:
            il = blk.instructions
            out = []
            changed = False
            for ins in il:
                si = getattr(ins, "sync_info", None)
                waits = list(si.on_wait) if si is not None and si.on_wait else []
                if len(waits) > max_waits:
                    si.on_wait = waits[:max_waits]
                    for w in waits[max_waits:]:
                        counter += 1
                        out.append(
                            mybir.InstNoOp(
                                name=f"waitsplit_{counter}",
                                engine=ins.engine,
                                bass_nofuse=True,
                                sync_info=mybir.SyncInfo(on_wait=[w], on_update=[]),
                            )
                        )
                    changed = True
                out.append(ins)
            if changed:
                il.clear()
                il.extend(out)
    return counter


def _build_program(repeat=1, hw_loop=0, parts="full"):
    nc = bass.Bass()

    x_d = nc.declare_dram_parameter("x", [L, D], F32R, isOutput=False)
    wq_d = nc.declare_dram_parameter("wq", [D, DG], F32R, isOutput=False)
    wk_d = nc.declare_dram_parameter("wk", [D, DG], F32R, isOutput=False)
    wv_d = nc.declare_dram_parameter("wv", [D, DG], F32R, isOutput=False)
    qc_d = nc.declare_dram_parameter("qcross", [128, GC], F32R, isOutput=False)
    bq_d = nc.declare_dram_parameter("bq", [128, GC], F32, isOutput=False)
    bk_d = nc.declare_dram_parameter("bk", [128, GC], F32, isOutput=False)
    bv_d = nc.declare_dram_parameter("bv", [1, DG], F32, isOutput=False)
    mk_d = nc.declare_dram_parameter("maskm", [128, LC], F32, isOutput=False)
    out_d = nc.declare_dram_parameter("out", [L, DG], F32, isOutput=True)

    with tile.TileContext(nc, pool_alloc_mode="queue") as tc:
        with (
            tc.tile_pool(name="const", bufs=1) as const,
            tc.tile_pool(name="qkv", bufs=1) as qkv,
            tc.tile_pool(name="xt", bufs=1) as xtp,
            tc.tile_pool(name="wqk", bufs=1) as wqkp,
            tc.tile_pool(name="pt", bufs=2) as ptp,
            tc.tile_pool(name="fin", bufs=1) as finp,
            tc.tile_pool(name="ost", bufs=4) as ostp,
        ):
            # ---- constants ----
            ident_f = const.tile([128, 128], F32)
            make_identity(nc, ident_f[:])
            ident = const.tile([128, 128], F32R)
            nc.vector.tensor_copy(ident[:], ident_f[:])
            ones1 = const.tile([1, 128], F32)
            nc.vector.memset(ones1[:], 1.0)
            ones8 = const.tile([128, 8], F32)
            nc.vector.memset(ones8[:], 1.0)
            qc_sb = const.tile([128, GC], F32R)
            nc.sync.dma_start(out=qc_sb[:], in_=qc_d[:, :])
            bq_sb = const.tile([128, GC], F32)
            nc.sync.dma_start(out=bq_sb[:], in_=bq_d[:, :])
            bk_sb = const.tile([128, GC], F32)
            nc.sync.dma_start(out=bk_sb[:], in_=bk_d[:, :])
            bv_sb = const.tile([1, DG], F32)
            nc.sync.dma_start(out=bv_sb[:], in_=bv_d[:, :])
            mk_sb = const.tile([128, LC], F32)
            nc.sync.dma_start(out=mk_sb[:], in_=mk_d[:, :])
            bias_v = const.tile([128, DG], F32)

            def body():
                vt = [
                    qkv.tile([128, 8 * HS], F32R, tag=f"v{lc}", name=f"v{lc}")
                    for lc in range(LC)
                ]
                xT = [
                    xtp.tile([128, L], F32R, tag=f"xT{dc}", name=f"xT{dc}")
                    for dc in range(DC)
                ]
                ctxT = [
                    finp.tile([128, L], F32R, tag=f"ctxT{p}", name=f"ctxT{p}")
                    for p in range(GC)
                ]
                # denominator rows at 32-aligned partitions; pair p uses rows
                # 64*(p%2) and 64*(p%2)+32 (adjacent pairs use disjoint rows,
                # p and p+2 reuse rows after fin(p) has consumed them)
                den = finp.tile([97, L], F32, tag="den")
                nc.vector.memset(den[:], 1.0)  # rows read before all written
                # sel[97,4]: column j selects row 32*j
                sel = const.tile([97, 4], F32)
                nc.vector.memset(sel[:], 0.0)
                nc.vector.memset(sel[0:1, 0:1], 1.0)
                nc.vector.memset(sel[32:33, 1:2], 1.0)
                nc.vector.memset(sel[64:65, 2:3], 1.0)
                nc.vector.memset(sel[96:97, 3:4], 1.0)
                rcpT = [
                    finp.tile([128, 2 * LC], F32, tag=f"rcp{p}", name=f"rcp{p}")
                    for p in range(GC)
                ]

                # ---- phase A: load x, transpose to xT [d-part, l-free] ----
                with (
                    tc.tile_pool(name="stage", bufs=2) as stpool,
                    tc.tile_pool(name="psA", bufs=1, space="PSUM") as psA,
                ):
                    with nc.named_scope("xtranspose"):
                        for g in range(4):  # groups of 4 l-chunks
                            stages = []
                            for j in range(4):
                                lc = g * 4 + j
                                st = stpool.tile([128, D], F32R, tag="stage")
                                nc.sync.dma_start(
                                    out=st[:], in_=x_d[lc * 128:(lc + 1) * 128, :]
                                )
                                stages.append(st)
                            for dc in range(DC):
                                ps = psA.tile([128, 512], F32R, tag=f"xtps{dc}")
                                for j in range(4):
                                    nc.tensor.transpose(
                                        ps[:, j * 128:(j + 1) * 128],
                                        stages[j][:, dc * 128:(dc + 1) * 128],
                                        ident[:],
                                    )
                                if dc % 2 == 0:
                                    nc.vector.tensor_copy(
                                        xT[dc][:, g * 512:(g + 1) * 512], ps[:]
                                    )
                                else:
                                    nc.scalar.copy(
                                        xT[dc][:, g * 512:(g + 1) * 512], ps[:]
                                    )

                # ---- phases B+C interleaved; one shared PSUM pool (8 banks) ----
                with tc.tile_pool(name="psBC", bufs=1, space="PSUM") as psBC:
                    with nc.named_scope("biasv"):
                        psb = psBC.tile([128, 512], F32, tag="proj", bufs=2)
                        nc.tensor.matmul(
                            psb[:], ones1[:], bv_sb[:], start=True, stop=True
                        )
                        nc.vector.tensor_copy(bias_v[:], psb[:])

                    def proj_qk(wd, bias_sb, p, tagname):
                        # one dh-chunk projection into a cycled [128, L] tile
                        dst = qkv.tile(
                            [128, L], F32R, tag=tagname, bufs=2, name=f"{tagname}{p}"
                        )
                        wts = []
                        for dc in range(DC):
                            wt = wqkp.tile(
                                [128, 128], F32R, tag="wqk", bufs=8,
                                name=f"w_{tagname}_{p}_{dc}",
                            )
                            nc.sync.dma_start(
                                out=wt[:],
                                in_=wd[dc * 128:(dc + 1) * 128, p * 128:(p + 1) * 128],
                            )
                            wts.append(wt)
                        for w in range(4):  # lq windows of 512
                            psw = psBC.tile([128, 512], F32, tag="proj", bufs=2)
                            for dc in range(DC):
                                nc.tensor.matmul(
                                    psw[:],
                                    wts[dc][:],
                                    xT[dc][:, w * 512:(w + 1) * 512],
                                    start=(dc == 0),
                                    stop=(dc == DC - 1),
                                )
                            nc.vector.tensor_scalar_add(
                                dst[:, w * 512:(w + 1) * 512],
                                psw[:],
                                bias_sb[:, p:p + 1],
                            )
                        return dst

                    def proj_pair(p):
                        with nc.named_scope(f"proj{p}"):
                            q_t = proj_qk(wq_d, bq_sb, p, "qTs")
                            # q row-1 fix
                            nc.vector.tensor_copy(q_t[:, 1:2], qc_sb[:, p:p + 1])
                            k_t = proj_qk(wk_d, bk_sb, p, "kTs")
                        return q_t, k_t

                    def proj_v():
                        with nc.named_scope("proj_v"), tc.tile_pool(
                            name="wv", bufs=1
                        ) as wvp:
                            wv_t = []
                            for dc in range(DC):
                                wt = wvp.tile(
                                    [128, DG], F32R, tag="wv", bufs=8, name=f"wv{dc}"
                                )
                                nc.sync.dma_start(
                                    out=wt[:], in_=wv_d[dc * 128:(dc + 1) * 128, :]
                                )
                                wv_t.append(wt)
                            for lc in range(LC):
                                psv = psBC.tile([128, 512], F32, tag="proj", bufs=2)
                                for dc in range(DC):
                                    nc.tensor.matmul(
                                        psv[:],
                                        xT[dc][:, lc * 128:(lc + 1) * 128],
                                        wv_t[dc][:],
                                        start=(dc == 0),
                                        stop=(dc == DC - 1),
                                    )
                                v = vt[lc]
                                ones_cols = v.rearrange(
                                    "p (h s) -> p h s", s=HS
                                )[:, :, HD]
                                nc.vector.tensor_copy(ones_cols, ones8[:])
                                for h in range(8):
                                    nc.vector.tensor_add(
                                        v[:, h * HS:h * HS + HD],
                                        psv[:, h * HD:(h + 1) * HD],
                                        bias_v[:, h * HD:(h + 1) * HD],
                                    )

                    def scores(qk, p, q, c):
                        q_t, k_t = qk[p]
                        lq = q * 512
                        sAB = psBC.tile(
                            [128, 1024], F32, tag="sAB", bufs=2,
                            name=f"sAB_{p}_{q}_{c}",
                        )
                        nc.tensor.matmul(
                            sAB[:, 0:512],
                            k_t[0:64, c * 128:(c + 1) * 128],
                            q_t[0:64, lq:lq + 512],
                            start=True, stop=True,
                            tile_position=(0, 0),
                        )
                        nc.tensor.matmul(
                            sAB[:, 512:1024],
                            k_t[64:128, c * 128:(c + 1) * 128],
                            q_t[64:128, lq:lq + 512],
                            start=True, stop=True,
                            tile_position=(64, 0),
                        )
                        return sAB

                    def fin_pair(p):
                        with nc.named_scope(f"fin{p}"):
                            # transposed reciprocal denominators: rcpT[p][lq, 2*lc+h]
                            rt = psBC.tile(
                                [128, 2 * LC], F32, tag="proj", bufs=2,
                                name=f"rt{p}",
                            )
                            sc = 2 * (p % 2)
                            for lc in range(LC):
                                nc.tensor.transpose(
                                    rt[:, lc * 2:lc * 2 + 2],
                                    den[0:97, lc * 128:(lc + 1) * 128],
                                    sel[0:97, sc:sc + 2],
                                )
                            nc.vector.reciprocal(rcpT[p][:], rt[:])

                            for g in range(4):
                                fps = psBC.tile(
                                    [128, 512], F32R, tag="proj", bufs=2,
                                    name=f"fps{p}_{g}",
                                )
                                for j in range(4):
                                    lc = g * 4 + j
                                    nc.tensor.transpose(
                                        fps[:, j * 128:(j + 1) * 128],
                                        ctxT[p][:, lc * 128:(lc + 1) * 128],
                                        ident[:],
                                    )
                                for j in range(4):
                                    lc = g * 4 + j
                                    ost = ostp.tile(
                                        [128, 128], F32, tag="ost",
                                        name=f"ost_{p}_{lc}",
                                    )
                                    for h in range(2):
                                        nc.vector.tensor_scalar_mul(
                                            ost[:, h * 64:h * 64 + 64],
                                            fps[:, j * 128 + h * 64:j * 128 + h * 64 + 64],
                                            rcpT[p][:, lc * 2 + h:lc * 2 + h + 1],
                                        )
                                    nc.sync.dma_start(
                                        out=out_d[
                                            lc * 128:(lc + 1) * 128,
                                            p * 128:(p + 1) * 128,
                                        ],
                                        in_=ost[:],
                                    )

                    # ---- emission schedule: flat software pipeline ----
                    qk = [None] * GC
                    qk[0] = proj_pair(0)
                    proj_v()
                    qk[1] = proj_pair(1)

                    NSTEP = GC * 4 * LC  # (pair, quarter, chunk) steps
                    LOOKAHEAD = 2

                    def step_pqc(s_):
                        p_, r = divmod(s_, 4 * LC)
                        q_, c_ = divmod(r, LC)
                        return p_, q_, c_

                    sABs = {}
                    for s_ in range(LOOKAHEAD):
                        sABs[s_] = scores(qk, *step_pqc(s_))
                    cA = cB = None
                    for s_ in range(NSTEP):
                        p, q, c = step_pqc(s_)
                        if c == 0:
                            cA = psBC.tile([65, 512], F32, tag="ctxA",
                                           name=f"cA{p}_{q}")
                            cB = psBC.tile([65, 512], F32, tag="ctxB",
                                           name=f"cB{p}_{q}")
                        if s_ + LOOKAHEAD < NSTEP:
                            sABs[s_ + LOOKAHEAD] = scores(qk, *step_pqc(s_ + LOOKAHEAD))
                        sAB = sABs.pop(s_)
                        pt = ptp.tile([128, 1024], F32R, tag="pt")
                        nc.scalar.activation(
                            pt[:],
                            sAB[:],
                            mybir.ActivationFunctionType.Exp,
                            bias=mk_sb[:, c:c + 1],
                            scale=SCALE,
                        )
                        hA = 2 * p * HS
                        hB = (2 * p + 1) * HS
                        nc.tensor.matmul(
                            cA[:],
                            vt[c][:, hA:hA + HS],
                            pt[:, 0:512],
                            start=(c == 0), stop=(c == LC - 1),
                        )
                        nc.tensor.matmul(
                            cB[:],
                            vt[c][:, hB:hB + HS],
                            pt[:, 512:1024],
                            start=(c == 0), stop=(c == LC - 1),
                        )
                        if c == LC - 1:
                            lq = q * 512
                            nc.vector.tensor_copy(
                                ctxT[p][0:64, lq:lq + 512], cA[0:64, :]
                            )
                            nc.vector.tensor_copy(
                                ctxT[p][64:128, lq:lq + 512], cB[0:64, :]
                            )
                            pb = 64 * (p % 2)
                            nc.vector.tensor_copy(
                                den[pb:pb + 1, lq:lq + 512], cA[64:65, :]
                            )
                            nc.vector.tensor_copy(
                                den[pb + 32:pb + 33, lq:lq + 512], cB[64:65, :]
                            )
                            if q == 3:
                                if p + 2 < GC:
                                    qk[p + 2] = proj_pair(p + 2)
                                fin_pair(p)

            if hw_loop:
                with tc.For_i(0, hw_loop, 1):
                    body()
            else:
                for _rep in range(repeat):
                    body()

    _split_excess_waits(nc)
    return nc


def kernel(x, attn_mask, cross_cls_sent, Wq, bq, Wk, bk, Wv, bv):
    from concourse.bass_utils import run_bass_kernel_spmd

    x = np.asarray(x, dtype=np.float32)
    attn_mask = np.asarray(attn_mask, dtype=np.float32)
    cross = np.asarray(cross_cls_sent, dtype=np.float32)
    Wq = np.asarray(Wq, dtype=np.float32)
    bq = np.asarray(bq, dtype=np.float32)
    Wk = np.asarray(Wk, dtype=np.float32)
    bk = np.asarray(bk, dtype=np.float32)
    Wv = np.asarray(Wv, dtype=np.float32)
    bv = np.asarray(bv, dtype=np.float32)

    if "nc" not in _CACHED:
        _CACHED["nc"] = _build_program()
    nc = _CACHED["nc"]

    in_maps = []
    for c in range(NCORES):
        b = c // 2
        g = c % 2
        cols = slice(g * DG, (g + 1) * DG)
        qcross = cross[b] @ Wq[:, cols] + bq[cols]  # (512,) host matvec
        in_maps.append(
            {
                "x": np.ascontiguousarray(x[b]),
                "wq": np.ascontiguousarray(Wq[:, cols]),
                "wk": np.ascontiguousarray(Wk[:, cols]),
                "wv": np.ascontiguousarray(Wv[:, cols]),
                "qcross": np.ascontiguousarray(
                    qcross.reshape(GC, 128).T.astype(np.float32)
                ),
                "bq": np.ascontiguousarray(bq[cols].reshape(GC, 128).T),
                "bk": np.ascontiguousarray(bk[cols].reshape(GC, 128).T),
                "bv": np.ascontiguousarray(bv[cols].reshape(1, DG)),
                "maskm": np.ascontiguousarray(
                    attn_mask[b, 0, 0].reshape(LC, 128).T
                ),
            }
        )

    res = run_bass_kernel_spmd(nc, in_maps, list(range(NCORES)))
    out = np.empty((B, L, D), dtype=np.float32)
    for c in range(NCORES):
        b = c // 2
        g = c % 2
        out[b][:, g * DG:(g + 1) * DG] = res.results[c]["out"]
    return out

